# revision 57
# baseline (speedup 1.0000x reference)
"""Biaffine labeler kernel for 8 Trainium2 NeuronCores.

Computation (full shapes):
    dep  [2, 2048, 1024], head [2, 2049, 1024], head_indices [2, 2048]
    dep_label  = dep @ dep_W.T + dep_b                    [2, 2048, 512]
    selected   = (head gathered at head_indices) @ head_W.T + head_b
    logits[b,t,n] = dep_label[b,t,:] @ W[n] @ selected[b,t,:] + bias[n]

Sharding: data-parallel over (b, t): core c handles b = c // 4 and the
512-token range starting at (c % 4) * 512.  W / projections replicated.

Host prep: the head-row gather runs on the host (head_indices is
known; picking the rows each core needs is part of the sharding), and
all matmul inputs are pre-cast to bf16 and pre-tiled into device
layout, including W (26 MB bf16 instead of 52 MB fp32 through SWDGE).
Projection/label biases ship pre-transposed/pre-broadcast so no matmul
is spent on them.

Per-core device program:
    1. HWDGE input DMAs on the two rings: dep/depW interleaved in
       quarters on ring 1 in exactly projection consumption order,
       sel/headW in halves on ring 2
    2. ~3.4us of junk matmuls at the start warm the PE HAM clock gate
       (1.2 -> 2.4 GHz) while the first DMAs land
    3. projection j-blocks on PE interleaved (dep j01, j23, head j0-3,
       dep j45, head j4-7, dep j67) to match DMA arrival, drained with
       the bias fused: scalar ACT per-partition add for dep_labelT
       [512e, 512t], DVE tensor_add of a pre-broadcast row bias for
       selected [512t, 512e]
    4. biaffine: labels in groups (2,2,4,4,...); for each (group,
       token-chunk), the j-loop streams W[n] chunks against the SAME
       stationary dep_labelT chunk for the group's consecutive matmuls,
       and a post-pass (_dedupe_ldweights) removes the redundant
       LDWEIGHTS bass emits, keeping the matmul cadence at the N=512
       bf16 issue floor (~216 ns warm); A_n = dep_label @ W[n]
       accumulates into one PSUM bank per label, 8-bank ping-pong
       across token chunks
    5. DVE scalar_tensor_tensor + free-dim accumulator computes
       logits[t,n] = sum_e A_n[t,e]*sel[t,e] per (label, token chunk);
       label bias lands via 4 tensor_adds at the end, output DMA'd
       per token chunk
    6. W streams via HWDGE with ONE DMA per label group (one completion
       semaphore, no per-group matmul hiccup), alternating rings; the
       two leading half-size groups land before the projections finish
       so the biaffine starts without waiting on W
"""

import sys

for _p in ("/opt/trn_rl_repo", "/root/.axon_site/_ro/trn_rl_repo"):
    if _p not in sys.path:
        sys.path.append(_p)

from contextlib import ExitStack

import ml_dtypes
import numpy as np

BF16NP = ml_dtypes.bfloat16

import concourse.mybir as mybir
import concourse.tile as tile
from concourse import bacc
from concourse.bass_utils import run_bass_kernel_spmd

import bass_rust as _bass_rust

B, T, D = 2, 2048, 1024
E = 512            # label-space dim (D // 2)
NLAB = 50
NCORES = 8
TLOC = (B * T) // NCORES   # 512 tokens per core
TP = TLOC // 128           # 4 token chunks
DP = D // 128              # 8 contraction chunks for the projections
EP = E // 128              # 4 chunks of the label dim

F32 = mybir.dt.float32
BF16 = mybir.dt.bfloat16

GROUP = 4                  # labels per PSUM group
WGBUFS = 4                 # W group tiles resident
LOOKAHEAD = 3              # W prefetch distance in groups


def _dedupe_ldweights(nc):
    """Remove LDWEIGHTS whose stationary AP equals the immediately
    preceding one.  With the label-inner matmul ordering, 4 consecutive
    matmuls share the stationary operand; bass emits one LDWEIGHTS per
    matmul unconditionally, and each costs ~46ns of PE issue time.
    Safe here because no SBUF region used as a stationary operand is
    ever rewritten.  Deps of a dropped LDWEIGHTS move to the next
    instruction (its matmul)."""
    for f in nc.m.functions:
        for blk in f.blocks:
            insts = blk.instructions
            last_sig = None
            newlist = []
            pending = None
            changed = False
            for inst in insts:
                if isinstance(inst, _bass_rust.InstLdweights):
                    sig = str(inst.ins[0]).split("bass_ap=")[0]
                    if sig == last_sig:
                        pending = inst
                        changed = True
                        continue
                    last_sig = sig
                if pending is not None:
                    inst.merge_dependencies_from(pending)
                    pending = None
                newlist.append(inst)
            if changed:
                del insts[:]
                insts.extend(newlist)


def build_program():
    nc = bacc.Bacc("TRN2", target_bir_lowering=False, debug=False,
                   num_devices=NCORES)

    dep_T = nc.dram_tensor("dep_T", [128, DP, TLOC], BF16,
                           kind="ExternalInput").ap()
    selT = nc.dram_tensor("selT", [128, DP, TLOC], BF16,
                          kind="ExternalInput").ap()
    depW_T = nc.dram_tensor("depW_T", [128, DP, E], BF16,
                            kind="ExternalInput").ap()
    headW_T = nc.dram_tensor("headW_T", [128, DP, E], BF16,
                             kind="ExternalInput").ap()
    depbT = nc.dram_tensor("depbT", [128, EP], F32,
                           kind="ExternalInput").ap()
    headbBC = nc.dram_tensor("headbBC", [128, E], BF16,
                             kind="ExternalInput").ap()
    biasBC = nc.dram_tensor("biasBC", [128, NLAB], F32,
                            kind="ExternalInput").ap()
    Wt = nc.dram_tensor("Wt", [NLAB, 128, EP, E], BF16,
                        kind="ExternalInput").ap()
    logits = nc.dram_tensor("logits", [TLOC, NLAB], F32,
                            kind="ExternalOutput").ap()

    with tile.TileContext(nc) as tc, ExitStack() as ctx:
        pp = ctx.enter_context(tc.tile_pool(name="persist", bufs=1))

        def ptile(shape, dtype, name):
            return pp.tile(shape, dtype, tag=name, name=name)

        ones_r = ptile([1, TLOC], BF16, "ones_r")
        depb_sb = ptile([128, EP], F32, "depb_sb")
        headb_bc = ptile([128, E], BF16, "headb_bc")
        bias_bc = ptile([128, NLAB], F32, "bias_bc")
        dep_sT = ptile([128, DP, TLOC], BF16, "dep_sT")   # [d, tok]
        sel_rT = ptile([128, DP, TLOC], BF16, "sel_rT")   # [d, tok]
        depWT = ptile([128, DP, E], BF16, "depWT")        # [d, e]
        headWT = ptile([128, DP, E], BF16, "headWT")      # [d, e]
        dep_lT = ptile([128, EP, TLOC], BF16, "dep_lT")   # [e, tok]
        sel_sb = ptile([128, TP, E], BF16, "sel_sb")      # [tok, e]
        logit_sb = ptile([128, TP, NLAB], F32, "logit_sb")
        logit_out = ptile([128, TP, NLAB], F32, "logit_out")

        # ---- input DMAs; dep/depW interleaved in quarters on one ring,
        # in exactly the order the dep projection consumes them ----
        QD = 2
        for q in range(0, DP, QD):
            nc.sync.dma_start(dep_sT[:, q:q + QD, :], dep_T[:, q:q + QD, :])
            nc.sync.dma_start(depWT[:, q:q + QD, :], depW_T[:, q:q + QD, :])
        HD = DP // 2
        for h in range(0, DP, HD):
            nc.scalar.dma_start(sel_rT[:, h:h + HD, :], selT[:, h:h + HD, :])
            nc.scalar.dma_start(headWT[:, h:h + HD, :],
                                headW_T[:, h:h + HD, :])
        nc.scalar.dma_start(depb_sb[:], depbT)
        nc.scalar.dma_start(headb_bc[:], headbBC)
        nc.scalar.dma_start(bias_bc[:], biasBC)
        nc.vector.memset(ones_r[:], 1.0)

        ps_pool = ctx.enter_context(
            tc.tile_pool(name="ps", bufs=8, space="PSUM"))

        # HAM warm-up: ~3.4us of junk matmuls while the input DMAs land,
        # so the projections start at the full 2.4 GHz PE clock
        psw = ps_pool.tile([128, 512], F32, tag="ps", name="psw")
        for w in range(8):
            nc.tensor.matmul(psw[:], ones_r[:, :128], ones_r[:],
                             start=True, stop=True)

        # projections, j-blocks interleaved to match DMA arrival order:
        # dep quarters land at ~11/14/16/19us on ring 1 while sel/headW
        # halves land at ~14/19us on ring 2
        dps = [ps_pool.tile([128, 512], F32, tag="ps", name=f"psd{i}")
               for i in range(EP)]
        hps = [ps_pool.tile([128, 512], F32, tag="ps", name=f"psh{i}")
               for i in range(TP)]

        def dep_block(js):
            for j in js:
                for i in range(EP):
                    nc.tensor.matmul(dps[i][:],
                                     depWT[:, j, i * 128:(i + 1) * 128],
                                     dep_sT[:, j, :],
                                     start=(j == 0), stop=(j == DP - 1))

        def head_block(js):
            for j in js:
                for i in range(TP):
                    nc.tensor.matmul(hps[i][:],
                                     sel_rT[:, j, i * 128:(i + 1) * 128],
                                     headWT[:, j, :],
                                     start=(j == 0), stop=(j == DP - 1))

        dep_block((0, 1))
        dep_block((2, 3))
        head_block((0, 1, 2, 3))
        dep_block((4, 5))
        head_block((4, 5, 6, 7))
        for i in range(TP):
            nc.vector.tensor_add(sel_sb[:, i, :], hps[i][:], headb_bc[:])
        dep_block((6, 7))
        for i in range(EP):
            nc.scalar.add(dep_lT[:, i, :], dps[i][:], depb_sb[:, i:i + 1])

        # ---- biaffine main loop; W streamed one DMA per label group ----
        w_pool = ctx.enter_context(tc.tile_pool(name="wg", bufs=WGBUFS))
        dve_dead = ctx.enter_context(tc.tile_pool(name="dd", bufs=2))

        # first two groups are half-size so their W (1 MB each, one per
        # ring) lands before the projections finish; the tail group
        # absorbs the remainder
        sizes = [2, 2] + [GROUP] * ((NLAB - 4) // GROUP) + (
            [NLAB - 4 - GROUP * ((NLAB - 4) // GROUP)]
            if (NLAB - 4) % GROUP else [])
        groups = []
        s = 0
        for sz in sizes:
            groups.append(list(range(s, s + sz)))
            s += sz
        wgtiles = {}

        def fetch_group(gi):
            if gi >= len(groups):
                return
            glen = len(groups[gi])
            n0 = groups[gi][0]
            wg = w_pool.tile([128, GROUP, EP, E], BF16, tag="wg",
                             name=f"wg{gi}")
            eng = nc.sync if gi % 2 == 0 else nc.scalar
            eng.dma_start(wg[:, 0:glen, :, :],
                          Wt[n0:n0 + glen].rearrange("g p j e -> p g j e"))
            wgtiles[gi] = wg

        for gi in range(LOOKAHEAD):
            fetch_group(gi)

        logits_r = logits.rearrange("(i p) n -> p i n", p=128)
        for gi, grp in enumerate(groups):
            wg = wgtiles[gi]
            for i in range(TP):
                pss = [ps_pool.tile([128, 512], F32, tag="ps",
                                    name=f"ps_{gi}_{i}_{k}")
                       for k in range(len(grp))]
                for j in range(EP):
                    lhs = dep_lT[:, j, i * 128:(i + 1) * 128]
                    for k in range(len(grp)):
                        nc.tensor.matmul(pss[k][:], lhs,
                                         wg[:, k, j, :],
                                         start=(j == 0), stop=(j == EP - 1))
                for k, n in enumerate(grp):
                    dead = dve_dead.tile([128, E], BF16, tag="dd",
                                         name=f"dd_{gi}_{i}_{k}")
                    nc.vector.scalar_tensor_tensor(
                        out=dead[:], in0=pss[k][:], scalar=1.0,
                        in1=sel_sb[:, i, :],
                        op0=mybir.AluOpType.mult,
                        op1=mybir.AluOpType.mult,
                        accum_out=logit_sb[:, i, n:n + 1])
            fetch_group(gi + LOOKAHEAD)

        for i in range(TP):
            nc.vector.tensor_add(logit_out[:, i, :], logit_sb[:, i, :],
                                 bias_bc[:])
            nc.sync.dma_start(logits_r[:, i, :], logit_out[:, i, :])

    _dedupe_ldweights(nc)
    nc.compile()
    return nc


_NC_CACHE = []


def _get_program():
    if not _NC_CACHE:
        _NC_CACHE.append(build_program())
    return _NC_CACHE[0]


def _dev_layout(a):
    # [x, 1024] operand -> transposed bf16 tile layout [128, 8, x]
    at = np.asarray(a, dtype=np.float32).T.astype(BF16NP)
    return np.ascontiguousarray(
        at.reshape(DP, 128, at.shape[1]).transpose(1, 0, 2))


def make_in_maps(dep, head, head_indices, dep_W, dep_b, head_W, head_b, W,
                 bias):
    dep = np.asarray(dep, dtype=np.float32)
    head = np.asarray(head, dtype=np.float32)
    idx = np.asarray(head_indices)
    W = np.asarray(W, dtype=np.float32)
    depb = np.asarray(dep_b, dtype=np.float32)
    headb = np.asarray(head_b, dtype=np.float32)
    shared = {
        "depW_T": _dev_layout(dep_W),
        "headW_T": _dev_layout(head_W),
        "depbT": np.ascontiguousarray(depb.reshape(EP, 128).T),
        "headbBC": np.ascontiguousarray(
            np.broadcast_to(headb.astype(BF16NP), (128, E))),
        "biasBC": np.ascontiguousarray(np.broadcast_to(
            np.asarray(bias, dtype=np.float32), (128, NLAB))),
        "Wt": np.ascontiguousarray(
            W.reshape(NLAB, EP, 128, E).transpose(0, 2, 1, 3).astype(BF16NP)),
    }
    in_maps = []
    cores_per_b = NCORES // B
    for c in range(NCORES):
        b = c // cores_per_b
        t0 = (c % cores_per_b) * TLOC
        rows = head[b][idx[b, t0:t0 + TLOC]]        # host-side gather
        in_maps.append({
            "dep_T": _dev_layout(dep[b, t0:t0 + TLOC]),
            "selT": _dev_layout(rows),
            **shared,
        })
    return in_maps


def run_sharded(inputs, trace=False):
    """Run the SPMD kernel; returns (full_logits, BassKernelResults)."""
    nc = _get_program()
    in_maps = make_in_maps(
        inputs["dep"], inputs["head"], inputs["head_indices"],
        inputs["dep_W"], inputs["dep_b"], inputs["head_W"],
        inputs["head_b"], inputs["W"], inputs["bias"])
    for attempt in range(3):
        try:
            res = run_bass_kernel_spmd(nc, in_maps, list(range(NCORES)),
                                       trace=trace)
            break
        except Exception:  # transient NRT_EXEC device errors
            if attempt == 2:
                raise
            import time
            time.sleep(5)
    out = np.empty((B, T, NLAB), dtype=np.float32)
    cores_per_b = NCORES // B
    for c in range(NCORES):
        b = c // cores_per_b
        t0 = (c % cores_per_b) * TLOC
        out[b, t0:t0 + TLOC] = np.asarray(res.results[c]["logits"],
                                          dtype=np.float32)
    return out, res


def kernel(dep, head, head_indices, mask, dep_W, dep_b, head_W, head_b, W,
           bias):
    out, _ = run_sharded({
        "dep": dep, "head": head, "head_indices": head_indices,
        "dep_W": dep_W, "dep_b": dep_b, "head_W": head_W,
        "head_b": head_b, "W": W, "bias": bias,
    })
    return out
